# revision 10
# baseline (speedup 1.0000x reference)
"""Trainium2 Bass kernel for CEN patch expert (im2col + patch-norm + 122-512-128-1 MLP).

Strategy (8 NeuronCores, data-parallel over batch B=32 -> 4 images/core):
  - Patch stats (mean/std over the 121 pixels of each 11x11 patch) computed
    separably from the image: vertical band-matmul + horizontal sliding-sum
    (log-shift adds on DVE), giving S = sum(p), Ssq = sum(p^2) per position.
  - Normalization folded into the first matmul:
        h1_pre = Wp @ (p * inv) - rowsum(Wp) * (mean * inv) + (W1[:,0] + b1)
    rhs rows = [p*inv (121); mean*inv (1); ones (1)]  (K = 123)
    lhsT rows = [Wp.T; -rowsum; W1[:,0]+b1]
  - im2col via overlapping-AP DMAs (11 per image) into a [122, 9409] SBUF
    buffer (row 121 = mean); inv broadcast per tile from a DRAM scratch row.
  - All MLP matmuls in float32r (full PE rate, ~1e-4 rel err).
  - tanh/sigmoid on ScalarE; MM3 col-tiled so 4 tiles share one sigmoid.
"""

import numpy as np

import concourse.bacc as bacc
import concourse.bass as bass
import concourse.tile as tile
import concourse.mybir as mybir
from concourse.bass_utils import run_bass_kernel_spmd

N_CORES = 8
B = 32
H = 107
PATCH = 11
R = 97          # output rows/cols
L = R * R       # 9409 positions per image
K = PATCH * PATCH  # 121
IPC = B // N_CORES  # images per core = 4
LP = L + 1      # padded positions (even last tile for fp32r matmul ISA)
NT = 512        # positions per tile
NTILES = (LP + NT - 1) // NT  # 19 (18x512 + 194)
GROUP = 4       # tiles per MM3/sigmoid group (col-tiling)

F32 = mybir.dt.float32
F32R = mybir.dt.float32r
Tanh = mybir.ActivationFunctionType.Tanh
Sigmoid = mybir.ActivationFunctionType.Sigmoid
Sqrt = mybir.ActivationFunctionType.Sqrt


def build():
    nc = bacc.Bacc("TRN2", target_bir_lowering=False, debug=False,
                   num_devices=N_CORES)
    x4 = nc.dram_tensor("x4", (IPC, H, H), F32, kind="ExternalInput")
    w1e = nc.dram_tensor("w1e", (123, 512), F32R, kind="ExternalInput")
    w2t = nc.dram_tensor("w2t", (128, 512), F32R, kind="ExternalInput")
    w3t = nc.dram_tensor("w3t", (128, 1), F32R, kind="ExternalInput")
    b2c = nc.dram_tensor("b2c", (128, 1), F32, kind="ExternalInput")
    b3c = nc.dram_tensor("b3c", (1, 1), F32, kind="ExternalInput")
    av = nc.dram_tensor("av", (H, R), F32, kind="ExternalInput")
    y4 = nc.dram_tensor("y4", (IPC, L), F32, kind="ExternalOutput")
    invflat = nc.dram_tensor("invflat", (IPC, LP), F32, kind="Internal")

    xt = x4.ap().tensor
    invt_d = invflat.ap().tensor

    with tile.TileContext(nc) as tc:
        with (
            tc.tile_pool(name="wp", bufs=1) as wp,
            tc.tile_pool(name="stat", bufs=1) as st,
            tc.tile_pool(name="pim", bufs=2) as pim,
            tc.tile_pool(name="bcp", bufs=3) as bcp,
            tc.tile_pool(name="rhp", bufs=3) as rhp,
            tc.tile_pool(name="h1p", bufs=2) as h1p,
            tc.tile_pool(name="h2p", bufs=2) as h2p,
            tc.tile_pool(name="outp", bufs=3) as outp,
            tc.tile_pool(name="pg", bufs=3, space="PSUM") as pg,
            tc.tile_pool(name="ps2", bufs=1, space="PSUM") as ps2p,
            tc.tile_pool(name="ps3", bufs=1, space="PSUM") as ps3p,
        ):
            # ---- weights / consts ----
            w1s = wp.tile([123, 512], F32R, tag="w1s")
            nc.sync.dma_start(out=w1s, in_=w1e.ap()[:, :])
            w2s = wp.tile([128, 512], F32R, tag="w2s")
            nc.sync.dma_start(out=w2s, in_=w2t.ap()[:, :])
            w3s = wp.tile([128, 1], F32R, tag="w3s")
            nc.sync.dma_start(out=w3s, in_=w3t.ap()[:, :])
            b2s = wp.tile([128, 1], F32, tag="b2s")
            nc.sync.dma_start(out=b2s, in_=b2c.ap()[:, :])
            b3s = wp.tile([GROUP, 1], F32, tag="b3s")
            nc.sync.dma_start(
                out=b3s,
                in_=bass.AP(tensor=b3c.ap().tensor, offset=0,
                            ap=[[0, GROUP], [1, 1]]))
            avs = wp.tile([H, R], F32, tag="avs")
            nc.sync.dma_start(out=avs, in_=av.ap()[:, :])
            onesf = wp.tile([1, NT], F32, tag="onesf")
            nc.vector.memset(onesf, 1.0)
            onesr = wp.tile([1, NT], F32R, tag="onesr")
            nc.vector.tensor_copy(onesr, onesf)

            # ---- Phase A: per-position patch stats for all 4 images ----
            # xall[r, img, c] = x4[img, r, c]
            xall = st.tile([H, IPC, H], F32, tag="xall")
            nc.sync.dma_start(
                out=xall,
                in_=bass.AP(tensor=xt, offset=0,
                            ap=[[H, H], [H * H, IPC], [1, H]]))
            xsq = st.tile([H, IPC, H], F32, tag="xsq")
            nc.vector.tensor_mul(xsq, xall, xall)

            # vertical band sums: V[i, img, c] = sum_kh x[i+kh, img, c]
            W4 = IPC * H  # 428
            vps = ps2p.tile([128, NT], F32, tag="s2")
            vsqps = ps3p.tile([128, NT], F32, tag="s3")
            for img in range(IPC):
                nc.tensor.matmul(vps[0:R, img * H:(img + 1) * H],
                                 lhsT=avs, rhs=xall[:, img, :],
                                 start=True, stop=True)
                nc.tensor.matmul(vsqps[0:R, img * H:(img + 1) * H],
                                 lhsT=avs, rhs=xsq[:, img, :],
                                 start=True, stop=True)
            vv = st.tile([R, 2 * W4], F32, tag="vv")  # [97, 856]: V | Vsq
            nc.vector.tensor_copy(vv[:, 0:W4], vps[0:R, 0:W4])
            nc.vector.tensor_copy(vv[:, W4:2 * W4], vsqps[0:R, 0:W4])

            # horizontal sliding sum of 11 via log-shift adds.
            # segment layout: 8 segments of width 107 at offsets k*107.
            M = 2 * W4  # 856
            w2v = st.tile([R, M - 1], F32, tag="w2v")
            nc.vector.tensor_add(w2v, vv[:, 0:M - 1], vv[:, 1:M])
            w4v = st.tile([R, M - 3], F32, tag="w4v")
            nc.vector.tensor_add(w4v, w2v[:, 0:M - 3], w2v[:, 2:M - 1])
            w8v = st.tile([R, M - 7], F32, tag="w8v")
            nc.vector.tensor_add(w8v, w4v[:, 0:M - 7], w4v[:, 4:M - 3])
            tv = st.tile([R, M - 10], F32, tag="tv")
            nc.vector.tensor_add(tv, w8v[:, 0:M - 10], w2v[:, 8:M - 2])
            sv = st.tile([R, M - 10], F32, tag="sv")  # [97, 846]
            nc.vector.tensor_add(sv, tv, vv[:, 10:M])

            def seg_view(base_off):
                # [97, IPC, 97] view into sv at given flat offset
                return bass.AP(tensor=sv.tensor, offset=sv.offset + base_off,
                               ap=[sv.ap[0], [H, IPC], [1, R]])

            Sview = seg_view(0)
            Qview = seg_view(W4)

            t1 = st.tile([R, IPC, R], F32, tag="t1")
            nc.vector.tensor_mul(t1, Sview, Sview)
            u = st.tile([R, IPC, R], F32, tag="u")
            # u = Ssq - S^2/121
            nc.vector.scalar_tensor_tensor(
                out=u, in0=t1, scalar=-1.0 / K, in1=Qview,
                op0=mybir.AluOpType.mult, op1=mybir.AluOpType.add)
            stdt = st.tile([R, IPC, R], F32, tag="stdt")
            # std = sqrt(u / 120)
            nc.scalar.activation(out=stdt, in_=u, func=Sqrt,
                                 bias=0.0, scale=1.0 / (K - 1))
            invs = st.tile([R, IPC, R], F32, tag="invs")
            nc.vector.reciprocal(invs, stdt)
            meant = st.tile([R, IPC, R], F32, tag="meant")
            nc.vector.tensor_scalar_mul(meant, Sview, 1.0 / K)
            for img in range(IPC):
                nc.sync.dma_start(
                    out=bass.AP(tensor=invt_d, offset=img * LP,
                                ap=[[R, R], [1, R]]),
                    in_=invs[:, img, :])
                nc.sync.dma_start(
                    out=bass.AP(tensor=invt_d, offset=img * LP + L,
                                ap=[[1, 1], [1, 1]]),
                    in_=onesf[0:1, 0:1])

            # ---- Phase B: im2col + MLP per image ----
            for img in range(IPC):
                pimg = pim.tile([122, LP], F32, tag="pimg")
                nc.vector.memset(pimg[:, L:LP], 0.0)
                for kh in range(PATCH):
                    nc.sync.dma_start(
                        out=pimg[kh * PATCH:(kh + 1) * PATCH, 0:L]
                            .rearrange("p (i j) -> p i j", i=R),
                        in_=bass.AP(tensor=xt, offset=img * H * H + kh * H,
                                    ap=[[1, PATCH], [H, R], [1, R]]))
                # mean row
                nc.sync.dma_start(
                    out=pimg[121:122, 0:L].rearrange("p (i j) -> p i j", i=R),
                    in_=meant[:, img, :])

                for t in range(NTILES):
                    n0 = t * NT
                    nt = min(NT, LP - n0)
                    nt_out = min(nt, L - n0)
                    bc = bcp.tile([122, NT], F32, tag="bc")
                    nc.sync.dma_start(
                        out=bc[:, 0:nt],
                        in_=bass.AP(tensor=invt_d, offset=img * LP + n0,
                                    ap=[[0, 122], [1, nt]]))
                    rhs = rhp.tile([123, NT], F32R, tag="rhs")
                    nc.vector.tensor_mul(rhs[0:122, 0:nt],
                                         pimg[:, n0:n0 + nt],
                                         bc[:, 0:nt])
                    nc.sync.dma_start(out=rhs[122:123, 0:nt],
                                      in_=onesr[:, 0:nt])
                    h1 = h1p.tile([128, 4, NT], F32R, tag="h1")
                    for gg in range(2):
                        pgt = pg.tile([128, 1024], F32, tag="g")
                        for c in range(2):
                            mc = gg * 2 + c
                            nc.tensor.matmul(
                                pgt[:, c * NT:c * NT + nt],
                                lhsT=w1s[:, mc * 128:(mc + 1) * 128],
                                rhs=rhs[:, 0:nt],
                                start=True, stop=True)
                        nc.scalar.activation(
                            out=h1[:, 2 * gg:2 * gg + 2, 0:nt],
                            in_=pgt.rearrange("p (c n) -> p c n", c=2)[:, :, 0:nt],
                            func=Tanh)
                    ps2 = ps2p.tile([128, NT], F32, tag="s2")
                    for c in range(4):
                        nc.tensor.matmul(
                            ps2[:, 0:nt],
                            lhsT=w2s[:, c * 128:(c + 1) * 128],
                            rhs=h1[:, c, 0:nt],
                            start=(c == 0), stop=(c == 3))
                    h2 = h2p.tile([128, NT], F32R, tag="h2")
                    nc.scalar.activation(out=h2[:, 0:nt], in_=ps2[:, 0:nt],
                                         func=Tanh, bias=b2s[:, 0:1])
                    ps3 = ps3p.tile([128, NT], F32, tag="s3")
                    nc.tensor.matmul(ps3[0:1, 0:nt], lhsT=w3s,
                                     rhs=h2[:, 0:nt], start=True, stop=True)
                    outs = outp.tile([1, NT], F32, tag="outs")
                    nc.scalar.activation(out=outs[:, 0:nt], in_=ps3[0:1, 0:nt],
                                         func=Sigmoid, bias=b3s[0:1, 0:1])
                    nc.scalar.dma_start(
                        out=bass.AP(tensor=y4.ap().tensor,
                                    offset=img * L + n0,
                                    ap=[[1, 1], [1, nt_out]]),
                        in_=outs[:, 0:nt_out])
    nc.compile()
    return nc


def prep_inputs(x, W1, b1, W2, b2, W3, b3):
    x = np.asarray(x, dtype=np.float32)
    W1 = np.asarray(W1, dtype=np.float32)
    b1 = np.asarray(b1, dtype=np.float32)
    W2 = np.asarray(W2, dtype=np.float32)
    b2 = np.asarray(b2, dtype=np.float32)
    W3 = np.asarray(W3, dtype=np.float32)
    b3 = np.asarray(b3, dtype=np.float32)

    Wp = W1[:, 1:]  # (512, 121)
    w1e = np.concatenate(
        [Wp.T, -Wp.sum(axis=1)[None, :], (W1[:, 0] + b1)[None, :]],
        axis=0).astype(np.float32)  # (123, 512)
    w2t = np.concatenate(
        [W2[:, c * 128:(c + 1) * 128].T for c in range(4)],
        axis=1).astype(np.float32)  # (128, 512)
    w3t = W3.T.astype(np.float32).copy()  # (128, 1)
    b2c = b2[:, None].astype(np.float32).copy()
    b3c = b3.reshape(1, 1).astype(np.float32).copy()
    av = np.zeros((H, R), dtype=np.float32)
    for i in range(R):
        av[i:i + PATCH, i] = 1.0

    shared = {"w1e": w1e, "w2t": w2t, "w3t": w3t,
              "b2c": b2c, "b3c": b3c, "av": av}
    in_maps = []
    for c in range(N_CORES):
        m = dict(shared)
        m["x4"] = np.ascontiguousarray(x[c * IPC:(c + 1) * IPC, 0])
        in_maps.append(m)
    return in_maps


_CACHE = {}


def kernel(x, W1, b1, W2, b2, W3, b3):
    nc = _CACHE.get("nc")
    if nc is None:
        nc = build()
        _CACHE["nc"] = nc
    in_maps = prep_inputs(x, W1, b1, W2, b2, W3, b3)
    res = run_bass_kernel_spmd(nc, in_maps, core_ids=list(range(N_CORES)))
    y = np.stack([res.results[c]["y4"] for c in range(N_CORES)])  # (8,4,L)
    return y.reshape(B, 1, R, R).astype(np.float32)


if __name__ == "__main__":
    rng = np.random.default_rng(0)
    inputs = {
        "x": rng.standard_normal((B, 1, H, H), dtype=np.float32),
        "W1": (rng.standard_normal((512, 122)) * 0.05).astype(np.float32),
        "b1": (rng.standard_normal((512,)) * 0.05).astype(np.float32),
        "W2": (rng.standard_normal((128, 512)) * 0.05).astype(np.float32),
        "b2": (rng.standard_normal((128,)) * 0.05).astype(np.float32),
        "W3": (rng.standard_normal((1, 128)) * 0.05).astype(np.float32),
        "b3": (rng.standard_normal((1,)) * 0.05).astype(np.float32),
    }
    out = kernel(**inputs)
    print(out.shape, out.dtype)


# revision 12
# speedup vs baseline: 1.0466x; 1.0466x over previous
"""Trainium2 Bass kernel for CEN patch expert (im2col + patch-norm + 122-512-128-1 MLP).

Strategy (8 NeuronCores, data-parallel over batch B=32 -> 4 images/core):
  - Patch stats (mean/std over the 121 pixels of each 11x11 patch) computed
    separably from the image: vertical band-matmul + horizontal sliding-sum
    (log-shift adds on DVE), giving S = sum(p), Ssq = sum(p^2) per position.
  - Normalization folded into the first matmul:
        h1_pre = Wp @ (p * inv) - rowsum(Wp) * (mean * inv) + (W1[:,0] + b1)
    rhs rows = [p*inv (121); mean*inv (1); std*inv = 1 (1)]  (K = 123)
    lhsT rows = [Wp.T; -rowsum; W1[:,0]+b1]
  - im2col via overlapping-AP DMAs (11 per image) into a [122, 9409] SBUF
    buffer (row 121 = mean); inv broadcast per tile from a DRAM scratch row.
  - All MLP matmuls in float32r (full PE rate, ~1e-4 rel err).
  - tanh/sigmoid on ScalarE; MM3 col-tiled so 4 tiles share one sigmoid.
"""

import numpy as np

import concourse.bacc as bacc
import concourse.bass as bass
import concourse.tile as tile
import concourse.mybir as mybir
from concourse.bass_utils import run_bass_kernel_spmd

N_CORES = 8
B = 32
H = 107
PATCH = 11
R = 97          # output rows/cols
L = R * R       # 9409 positions per image
K = PATCH * PATCH  # 121
IPC = B // N_CORES  # images per core = 4
LP = L + 1      # padded positions (even last tile for fp32r matmul ISA)
NT = 512        # positions per tile
NTILES = (LP + NT - 1) // NT  # 19 (18x512 + 194)
GROUP = 4       # tiles per MM3/sigmoid group (col-tiling)

F32 = mybir.dt.float32
F32R = mybir.dt.float32r
Tanh = mybir.ActivationFunctionType.Tanh
Sigmoid = mybir.ActivationFunctionType.Sigmoid
Sqrt = mybir.ActivationFunctionType.Sqrt


def build():
    nc = bacc.Bacc("TRN2", target_bir_lowering=False, debug=False,
                   num_devices=N_CORES)
    x4 = nc.dram_tensor("x4", (IPC, H, H), F32, kind="ExternalInput")
    w1e = nc.dram_tensor("w1e", (123, 512), F32R, kind="ExternalInput")
    w2t = nc.dram_tensor("w2t", (128, 512), F32R, kind="ExternalInput")
    w3t = nc.dram_tensor("w3t", (128, 1), F32R, kind="ExternalInput")
    b2c = nc.dram_tensor("b2c", (128, 1), F32, kind="ExternalInput")
    b3c = nc.dram_tensor("b3c", (1, 1), F32, kind="ExternalInput")
    av = nc.dram_tensor("av", (H, R), F32, kind="ExternalInput")
    y4 = nc.dram_tensor("y4", (IPC, L), F32, kind="ExternalOutput")
    invflat = nc.dram_tensor("invflat", (IPC, LP), F32, kind="Internal")

    xt = x4.ap().tensor
    invt_d = invflat.ap().tensor

    with tile.TileContext(nc) as tc:
        with (
            tc.tile_pool(name="wp", bufs=1) as wp,
            tc.tile_pool(name="stat", bufs=1) as st,
            tc.tile_pool(name="pim", bufs=2) as pim,
            tc.tile_pool(name="bcp", bufs=3) as bcp,
            tc.tile_pool(name="rhp", bufs=3) as rhp,
            tc.tile_pool(name="h1p", bufs=2) as h1p,
            tc.tile_pool(name="h2p", bufs=2) as h2p,
            tc.tile_pool(name="outp", bufs=2) as outp,
            tc.tile_pool(name="srp", bufs=2) as srp,
            tc.tile_pool(name="pg", bufs=2, space="PSUM") as pg,
        ):
            # ---- weights / consts ----
            w1s = wp.tile([123, 512], F32R, tag="w1s")
            nc.sync.dma_start(out=w1s, in_=w1e.ap()[:, :])
            w2s = wp.tile([128, 512], F32R, tag="w2s")
            nc.sync.dma_start(out=w2s, in_=w2t.ap()[:, :])
            w3s = wp.tile([128, 1], F32R, tag="w3s")
            nc.sync.dma_start(out=w3s, in_=w3t.ap()[:, :])
            b2s = wp.tile([128, 1], F32, tag="b2s")
            nc.sync.dma_start(out=b2s, in_=b2c.ap()[:, :])
            b3s = wp.tile([GROUP, 1], F32, tag="b3s")
            nc.sync.dma_start(
                out=b3s,
                in_=bass.AP(tensor=b3c.ap().tensor, offset=0,
                            ap=[[0, GROUP], [1, 1]]))
            avs = wp.tile([H, R], F32, tag="avs")
            nc.sync.dma_start(out=avs, in_=av.ap()[:, :])
            onesf = wp.tile([1, NT], F32, tag="onesf")
            nc.vector.memset(onesf, 1.0)

            # ---- Phase A: per-position patch stats for all 4 images ----
            # xall[r, img, c] = x4[img, r, c]
            xall = st.tile([H, IPC, H], F32, tag="xall")
            nc.sync.dma_start(
                out=xall,
                in_=bass.AP(tensor=xt, offset=0,
                            ap=[[H, H], [H * H, IPC], [1, H]]))
            xsq = st.tile([H, IPC, H], F32, tag="xsq")
            nc.vector.tensor_mul(xsq, xall, xall)

            # vertical band sums: V[i, img, c] = sum_kh x[i+kh, img, c]
            W4 = IPC * H  # 428
            vtile = pg.tile([128, 2048], F32, tag="g")
            for img in range(IPC):
                nc.tensor.matmul(vtile[0:R, img * H:(img + 1) * H],
                                 lhsT=avs, rhs=xall[:, img, :],
                                 start=True, stop=True)
                nc.tensor.matmul(vtile[0:R, 1024 + img * H:1024 + (img + 1) * H],
                                 lhsT=avs, rhs=xsq[:, img, :],
                                 start=True, stop=True)
            vv = st.tile([R, 2 * W4], F32, tag="vv")  # [97, 856]: V | Vsq
            nc.vector.tensor_copy(vv[:, 0:W4], vtile[0:R, 0:W4])
            nc.vector.tensor_copy(vv[:, W4:2 * W4], vtile[0:R, 1024:1024 + W4])

            # horizontal sliding sum of 11 via log-shift adds.
            # segment layout: 8 segments of width 107 at offsets k*107.
            M = 2 * W4  # 856
            w2v = st.tile([R, M - 1], F32, tag="w2v")
            nc.vector.tensor_add(w2v, vv[:, 0:M - 1], vv[:, 1:M])
            w4v = st.tile([R, M - 3], F32, tag="w4v")
            nc.vector.tensor_add(w4v, w2v[:, 0:M - 3], w2v[:, 2:M - 1])
            w8v = st.tile([R, M - 7], F32, tag="w8v")
            nc.vector.tensor_add(w8v, w4v[:, 0:M - 7], w4v[:, 4:M - 3])
            tv = st.tile([R, M - 10], F32, tag="tv")
            nc.vector.tensor_add(tv, w8v[:, 0:M - 10], w2v[:, 8:M - 2])
            sv = st.tile([R, M - 10], F32, tag="sv")  # [97, 846]
            nc.vector.tensor_add(sv, tv, vv[:, 10:M])

            def seg_view(base_off):
                # [97, IPC, 97] view into sv at given flat offset
                return bass.AP(tensor=sv.tensor, offset=sv.offset + base_off,
                               ap=[sv.ap[0], [H, IPC], [1, R]])

            Sview = seg_view(0)
            Qview = seg_view(W4)

            t1 = st.tile([R, IPC, R], F32, tag="t1")
            nc.vector.tensor_mul(t1, Sview, Sview)
            u = st.tile([R, IPC, R], F32, tag="u")
            # u = Ssq - S^2/121
            nc.vector.scalar_tensor_tensor(
                out=u, in0=t1, scalar=-1.0 / K, in1=Qview,
                op0=mybir.AluOpType.mult, op1=mybir.AluOpType.add)
            stdt = st.tile([R, IPC, R], F32, tag="stdt")
            # std = sqrt(u / 120)
            nc.scalar.activation(out=stdt, in_=u, func=Sqrt,
                                 bias=0.0, scale=1.0 / (K - 1))
            invs = st.tile([R, IPC, R], F32, tag="invs")
            nc.vector.reciprocal(invs, stdt)
            meant = st.tile([R, IPC, R], F32, tag="meant")
            nc.vector.tensor_scalar_mul(meant, Sview, 1.0 / K)
            for img in range(IPC):
                nc.sync.dma_start(
                    out=bass.AP(tensor=invt_d, offset=img * LP,
                                ap=[[R, R], [1, R]]),
                    in_=invs[:, img, :])
                nc.sync.dma_start(
                    out=bass.AP(tensor=invt_d, offset=img * LP + L,
                                ap=[[1, 1], [1, 1]]),
                    in_=onesf[0:1, 0:1])

            # ---- Phase B: im2col + MLP per image ----
            for img in range(IPC):
                pimg = pim.tile([123, LP], F32, tag="pimg")
                nc.vector.memset(pimg[:, L:LP], 0.0)
                for kh in range(PATCH):
                    nc.sync.dma_start(
                        out=pimg[kh * PATCH:(kh + 1) * PATCH, 0:L]
                            .rearrange("p (i j) -> p i j", i=R),
                        in_=bass.AP(tensor=xt, offset=img * H * H + kh * H,
                                    ap=[[1, PATCH], [H, R], [1, R]]))
                # mean row (121) and std row (122; std*inv = 1 in rhs)
                nc.sync.dma_start(
                    out=pimg[121:122, 0:L].rearrange("p (i j) -> p i j", i=R),
                    in_=meant[:, img, :])
                nc.sync.dma_start(
                    out=pimg[122:123, 0:L].rearrange("p (i j) -> p i j", i=R),
                    in_=stdt[:, img, :])

                ngroups = (NTILES + GROUP - 1) // GROUP
                for g in range(ngroups):
                    t0 = g * GROUP
                    gsz = min(GROUP, NTILES - t0)
                    srow = srp.tile([1, GROUP * NT], F32, tag="srow")
                    scols = 0
                    sc_list = []
                    for j in range(gsz):
                        t = t0 + j
                        n0 = t * NT
                        nt = min(NT, LP - n0)
                        bc = bcp.tile([123, NT], F32, tag="bc")
                        nc.sync.dma_start(
                            out=bc[:, 0:nt],
                            in_=bass.AP(tensor=invt_d, offset=img * LP + n0,
                                        ap=[[0, 123], [1, nt]]))
                        rhs = rhp.tile([123, NT], F32R, tag="rhs")
                        nc.vector.tensor_mul(rhs[:, 0:nt],
                                             pimg[:, n0:n0 + nt],
                                             bc[:, 0:nt])
                        h1 = h1p.tile([128, 4, NT], F32R, tag="h1")
                        gt = pg.tile([128, 2048], F32, tag="g")
                        for c in range(4):
                            nc.tensor.matmul(
                                gt[:, c * NT:c * NT + nt],
                                lhsT=w1s[:, c * 128:(c + 1) * 128],
                                rhs=rhs[:, 0:nt],
                                start=True, stop=True)
                        nc.scalar.activation(
                            out=h1[:, :, 0:nt],
                            in_=gt.rearrange("p (c n) -> p c n", c=4)[:, :, 0:nt],
                            func=Tanh)
                        s23 = pg.tile([128, 2048], F32, tag="g")
                        for c in range(4):
                            nc.tensor.matmul(
                                s23[:, 0:nt],
                                lhsT=w2s[:, c * 128:(c + 1) * 128],
                                rhs=h1[:, c, 0:nt],
                                start=(c == 0), stop=(c == 3))
                        h2 = h2p.tile([128, NT], F32R, tag="h2")
                        nc.scalar.activation(out=h2[:, 0:nt],
                                             in_=s23[:, 0:nt],
                                             func=Tanh, bias=b2s[:, 0:1])
                        nc.tensor.matmul(s23[0:1, 1024:1024 + nt],
                                         lhsT=w3s, rhs=h2[:, 0:nt],
                                         start=True, stop=True)
                        nc.vector.tensor_copy(
                            srow[0:1, scols:scols + nt],
                            s23[0:1, 1024:1024 + nt])
                        sc_list.append((n0, nt, min(nt, L - n0)))
                        scols += nt
                    outs = outp.tile([1, GROUP * NT], F32, tag="outs")
                    nc.scalar.activation(out=outs[0:1, 0:scols],
                                         in_=srow[0:1, 0:scols],
                                         func=Sigmoid, bias=b3s[0:1, 0:1])
                    base = t0 * NT
                    nout = sum(o for (_, _, o) in sc_list)
                    nc.gpsimd.dma_start(
                        out=bass.AP(tensor=y4.ap().tensor,
                                    offset=img * L + base,
                                    ap=[[1, 1], [1, nout]]),
                        in_=outs[0:1, 0:nout])
    nc.compile()
    return nc


def prep_inputs(x, W1, b1, W2, b2, W3, b3):
    x = np.asarray(x, dtype=np.float32)
    W1 = np.asarray(W1, dtype=np.float32)
    b1 = np.asarray(b1, dtype=np.float32)
    W2 = np.asarray(W2, dtype=np.float32)
    b2 = np.asarray(b2, dtype=np.float32)
    W3 = np.asarray(W3, dtype=np.float32)
    b3 = np.asarray(b3, dtype=np.float32)

    Wp = W1[:, 1:]  # (512, 121)
    w1e = np.concatenate(
        [Wp.T, -Wp.sum(axis=1)[None, :], (W1[:, 0] + b1)[None, :]],
        axis=0).astype(np.float32)  # (123, 512)
    w2t = np.concatenate(
        [W2[:, c * 128:(c + 1) * 128].T for c in range(4)],
        axis=1).astype(np.float32)  # (128, 512)
    w3t = W3.T.astype(np.float32).copy()  # (128, 1)
    b2c = b2[:, None].astype(np.float32).copy()
    b3c = b3.reshape(1, 1).astype(np.float32).copy()
    av = np.zeros((H, R), dtype=np.float32)
    for i in range(R):
        av[i:i + PATCH, i] = 1.0

    shared = {"w1e": w1e, "w2t": w2t, "w3t": w3t,
              "b2c": b2c, "b3c": b3c, "av": av}
    in_maps = []
    for c in range(N_CORES):
        m = dict(shared)
        m["x4"] = np.ascontiguousarray(x[c * IPC:(c + 1) * IPC, 0])
        in_maps.append(m)
    return in_maps


_CACHE = {}


def kernel(x, W1, b1, W2, b2, W3, b3):
    nc = _CACHE.get("nc")
    if nc is None:
        nc = build()
        _CACHE["nc"] = nc
    in_maps = prep_inputs(x, W1, b1, W2, b2, W3, b3)
    res = run_bass_kernel_spmd(nc, in_maps, core_ids=list(range(N_CORES)))
    y = np.stack([res.results[c]["y4"] for c in range(N_CORES)])  # (8,4,L)
    return y.reshape(B, 1, R, R).astype(np.float32)


if __name__ == "__main__":
    rng = np.random.default_rng(0)
    inputs = {
        "x": rng.standard_normal((B, 1, H, H), dtype=np.float32),
        "W1": (rng.standard_normal((512, 122)) * 0.05).astype(np.float32),
        "b1": (rng.standard_normal((512,)) * 0.05).astype(np.float32),
        "W2": (rng.standard_normal((128, 512)) * 0.05).astype(np.float32),
        "b2": (rng.standard_normal((128,)) * 0.05).astype(np.float32),
        "W3": (rng.standard_normal((1, 128)) * 0.05).astype(np.float32),
        "b3": (rng.standard_normal((1,)) * 0.05).astype(np.float32),
    }
    out = kernel(**inputs)
    print(out.shape, out.dtype)
